# revision 48
# baseline (speedup 1.0000x reference)
"""CrossFocusedLinearAttention Trainium2 kernel.

Per-core computation (1 batch item per NeuronCore, 8 cores):
  q = relu(query @ Wq)/s; k = relu(key_in @ Wk)/s   (s = softplus(scale), folded
  into Wq/Wk columns on host; the +eps inside relu-out is dropped — its relative
  effect is ~1e-6, far below matmul rounding)
  focus(x) = x^3 * ||x|| / ||x^3||  per token (over all C channels)
  per head: kv = k_f^T v ; z = 1/(q_f . ksum); x = (q_f @ kv) * z
  out = x @ Wp + bp

All matmul operands are bfloat16 (PSUM accumulation stays fp32); rel err vs
the fp32 reference is ~8e-3 (validated in numpy), well under the 2e-2 gate.
bf16 runs the PE at 1 cycle/row at ANY moving width (fp32r needs >=256), so:
  - the per-head kv blocks use exact 129-wide moving operands
    (128 v-channels + 1 appended rk column => ksum rides the kv matmul,
    eliminating the separate ksum matmuls)
  - phase 2 applies the z-normalizer with a per-channel t tensor computed by a
    block-masked ksum stationary (KB), then a single DVE divide; the
    eps seed is dropped (min |t| over the real inputs is ~4e2 >> eps)
  - the focus renorm rq cancels out of x entirely; rk is folded into v

Layout strategy (all contractions on the partition dim, no on-device transposes):
  - host supplies query^T/key_in^T/value^T  [C, N] bf16
  - k, v are produced in natural [token, chan] layout (lhsT = key_in^T blocks)
  - q is produced transposed [chan, token]  (lhsT = Wq blocks)
  - kv+ksum accumulated in PSUM over all tokens; k-side focus renorm rk is
    applied to v instead of k
  - x^T = blockdiag(kv) matmul with q^3 moving; t^T = KB matmul with q^3
    moving; x_s = x^T / t^T on DVE; out^T = Wp matmul + bias on GPSIMD
  - output returned as out^T f32, transposed back on host
"""

import os
import sys

import numpy as np

sys.path.insert(0, "/opt/trn_rl_repo")

P = 128
C = 512
N = 4096
CT = C // P            # 4 channel tiles
NH = 8                 # heads
HD = C // NH           # 64 head dim
JBLK = 512             # phase-1 token chunk
JC = N // JBLK         # 8
JSUB = JBLK // P       # 4 token tiles per chunk
IBLK = 512             # phase-2 token chunk
ICN = N // IBLK        # 8
KVW = P + 1            # kv moving width: 128 v-cols + 1 rk col
NCORES = 8

_CACHE = {}


def _build_nc():
    import concourse.mybir as mybir
    import concourse.tile as tile
    from concourse import bacc
    from contextlib import ExitStack

    f32 = mybir.dt.float32
    bf = mybir.dt.bfloat16
    AF = mybir.ActivationFunctionType
    OP = mybir.AluOpType

    # Pin every ACTIVATE to natural_log_exp_and_others (contains relu,
    # square, ln, exp, identity, copy) — the default greedy set chooser
    # flip-flops between tables, costing ~1.3us per reload (18 reloads seen).
    class _BaccOneActTable(bacc.Bacc):
        def insert_act_table_loads(self):
            import bass_rust as _br
            from concourse.hw_specs import get_activation_tables
            has_activation = any(
                isinstance(i, mybir.InstActivation)
                for b in self.main_func.blocks
                for i in b.instructions
            )
            if not has_activation:
                return
            tables = [
                (n, (s if n == "natural_log_exp_and_others" else set()))
                for n, s in get_activation_tables(self.m.arch).items()
            ]
            _br.insert_act_table_loads(self, tables)

    nc = _BaccOneActTable("TRN2", target_bir_lowering=False, debug=False)

    qT = nc.declare_dram_parameter("qT", [C, N], bf, isOutput=False)
    kT = nc.declare_dram_parameter("kT", [C, N], bf, isOutput=False)
    vT = nc.declare_dram_parameter("vT", [C, N], bf, isOutput=False)
    Wq = nc.declare_dram_parameter("Wq", [C, C], bf, isOutput=False)
    Wk = nc.declare_dram_parameter("Wk", [C, C], bf, isOutput=False)
    Wv = nc.declare_dram_parameter("Wv", [C, C], bf, isOutput=False)
    Wp = nc.declare_dram_parameter("Wp", [C, C], bf, isOutput=False)
    bp_col = nc.declare_dram_parameter("bp_col", [P, CT], f32, isOutput=False)
    ones_sb_in = nc.declare_dram_parameter("ones_in", [P, CT], bf, isOutput=False)
    sel8 = nc.declare_dram_parameter("sel8", [NH, CT, P], bf, isOutput=False)
    outT = nc.declare_dram_parameter("outT", [C, N], f32, isOutput=True)

    # DRAM views: [C, X] -> [P, CT, X] (partition, c-tile, token)
    qT_v = qT.rearrange("(t p) n -> p t n", p=P)
    kT_v = kT.rearrange("(t p) n -> p t n", p=P)
    vT_v = vT.rearrange("(t p) n -> p t n", p=P)
    outT_v = outT.rearrange("(t p) n -> p t n", p=P)
    Wq_v = Wq.rearrange("(t p) n -> p t n", p=P)
    Wk_v = Wk.rearrange("(t p) n -> p t n", p=P)
    Wv_v = Wv.rearrange("(t p) n -> p t n", p=P)
    Wp_v = Wp.rearrange("(t p) n -> p t n", p=P)

    with ExitStack() as ctx:
        tc = ctx.enter_context(tile.TileContext(nc))

        # ---------- persistent SBUF ----------
        wpool = ctx.enter_context(tc.tile_pool(name="weights", bufs=1))
        wk = wpool.tile([P, CT, C], bf, tag="wk")
        wv = wpool.tile([P, CT, C], bf, tag="wv")
        wq = wpool.tile([P, CT, C], bf, tag="wq")
        wp = wpool.tile([P, CT, C], bf, tag="wp")
        bp_sb = wpool.tile([P, CT], f32, tag="bp")
        ones_sb = wpool.tile([P, CT], bf, tag="ones")
        sel_sb = wpool.tile([NH, CT, P], bf, tag="sel8")
        # phase-1-critical loads go on the scalar/gpsimd DMA queues so they
        # run in parallel with the sync-queue data loads; per-ct splits let
        # the first projection matmul start as soon as wk[ct0] lands.
        for ct in range(CT):
            nc.scalar.dma_start(wk[:, ct, :], Wk_v[:, ct, :])
            nc.gpsimd.dma_start(wv[:, ct, :], Wv_v[:, ct, :])
        nc.scalar.dma_start(ones_sb[:], ones_sb_in[:])

        # blockdiag kv + masked ksum (built in transition, used in phase 2)
        bdpool = ctx.enter_context(tc.tile_pool(name="bdkv", bufs=1))
        bd = [bdpool.tile([P, P], bf, tag=f"bd{t}", name=f"bd{t}")
              for t in range(CT)]
        m8 = bdpool.tile([P, CT, NH], bf, tag="m8")
        for t in range(CT):
            nc.vector.memset(bd[t][:], 0.0)
        nc.vector.memset(m8[:], 0.0)

        # chunk-0 q-side warmup tiles (produced in the phase-1 tail so the
        # PE stays busy while the last kv chain drains; consumed in phase 2)
        qwp = ctx.enter_context(tc.tile_pool(name="qwarm", bufs=1))
        qtile0 = qwp.tile([P, CT, IBLK], bf, tag="qt0")
        u3qs0 = [qwp.tile([P, IBLK], bf, tag=f"u3q0_{i}", name=f"u3q0_{i}")
                 for i in range(CT)]

        # ================= PHASE 1: k/v -> kv, ksum =================
        with ExitStack() as p1:
            kvpool = p1.enter_context(
                tc.tile_pool(name="kvps", bufs=1, space="PSUM"))
            kv_half = [kvpool.tile([P, 2 * KVW], f32, tag="kva", name="kva"),
                       kvpool.tile([P, 2 * KVW], f32, tag="kvb", name="kvb")]

            kpp = p1.enter_context(
                tc.tile_pool(name="p1kproj", bufs=2, space="PSUM"))
            vpp = p1.enter_context(
                tc.tile_pool(name="p1vproj", bufs=3, space="PSUM"))
            ldp = p1.enter_context(tc.tile_pool(name="p1ld", bufs=3))
            wkp = p1.enter_context(tc.tile_pool(name="p1work", bufs=4))
            u3pl = p1.enter_context(tc.tile_pool(name="p1u3", bufs=4))
            vxp = p1.enter_context(tc.tile_pool(name="p1vx", bufs=4))
            smp = p1.enter_context(tc.tile_pool(name="p1small", bufs=6))

            # software pipeline: the kv matmuls for tile j are emitted after
            # the k/v projection matmuls of tile j+1, so the PE queue head
            # never waits on the DVE/ACT chain that produces u3/v_ext.
            def kv_mms(st):
                u3_p, vx_p, first_p, last_p = st
                for ct in range(CT):
                    # one accumulation group per kv_half tile: start/stop
                    # only on one member (start zeroes the whole region)
                    osl = slice((ct % 2) * KVW, (ct % 2) * KVW + KVW)
                    nc.tensor.matmul(
                        kv_half[ct // 2][:, osl],
                        u3_p[:, ct * P:(ct + 1) * P], vx_p[:, ct, :],
                        start=(first_p and ct % 2 == 0),
                        stop=(last_p and ct % 2 == 1))

            pending = None
            ktile = vtile = None
            for tix in range(JC * JSUB):
                jc, jj = divmod(tix, JSUB)
                first = (tix == 0)
                last = (tix == JC * JSUB - 1)
                if jj == 0:
                    ktile = ldp.tile([P, CT, JBLK], bf, tag="kld")
                    vtile = ldp.tile([P, CT, JBLK], bf, tag="vld")
                    base = jc * JBLK
                    if jc == 0:
                        # interleave k/v quarter-loads so the first k-proj
                        # matmul starts after ~128KB instead of ~1MB
                        for q in range(JSUB):
                            qs = slice(q * P, (q + 1) * P)
                            nc.sync.dma_start(
                                ktile[:, :, qs], kT_v[:, :, qs])
                            nc.sync.dma_start(
                                vtile[:, :, qs], vT_v[:, :, qs])
                        nc.scalar.dma_start(wq[:], Wq_v[:])
                        nc.gpsimd.dma_start(wp[:], Wp_v[:])
                        nc.scalar.dma_start(bp_sb[:], bp_col[:])
                        nc.scalar.dma_start(sel_sb[:], sel8[:])
                    else:
                        nc.sync.dma_start(
                            ktile[:], kT_v[:, :, base:base + JBLK])
                        nc.sync.dma_start(
                            vtile[:], vT_v[:, :, base:base + JBLK])
                jsl = slice(jj * P, (jj + 1) * P)

                kps = kpp.tile([P, C], f32, tag="kproj")
                for ct in range(CT):
                    nc.tensor.matmul(
                        kps[:], ktile[:, ct, jsl], wk[:, ct, :],
                        start=(ct == 0), stop=(ct == CT - 1))
                vps = vpp.tile([P, CT, P], f32, tag="vproj")
                for ct in range(CT):
                    nc.tensor.matmul(
                        vps[:], vtile[:, ct, jsl], wv[:, ct, :],
                        start=(ct == 0), stop=(ct == CT - 1))
                if pending is not None:
                    kv_mms(pending)

                rlu = wkp.tile([P, C], bf, tag="rlu")
                nc.scalar.activation(rlu[:], kps[:], AF.Relu)
                # u2 = rlu^2, S2 = sum_c rlu^2 (per token)
                u2 = wkp.tile([P, C], bf, tag="u2")
                S2 = smp.tile([P, 1], f32, tag="s2")
                nc.scalar.activation(
                    u2[:], rlu[:], AF.Square, accum_out=S2[:])
                u3 = u3pl.tile([P, C], bf, tag="u3")
                nc.vector.tensor_tensor(u3[:], u2[:], rlu[:], OP.mult)
                # u6 scratch + S6 = sum_c u3^2
                u6 = wkp.tile([P, C], bf, tag="u6")
                S6 = smp.tile([P, 1], f32, tag="s6")
                nc.scalar.activation(
                    u6[:], u3[:], AF.Square, accum_out=S6[:])
                # rk = sqrt(S2/S6) = exp(0.5*ln(S2 * (1/S6)))
                rS6 = smp.tile([P, 1], f32, tag="rs6")
                nc.vector.reciprocal(rS6[:], S6[:])
                ratio = smp.tile([P, 1], f32, tag="ratio")
                nc.vector.tensor_tensor(ratio[:], S2[:], rS6[:], OP.mult)
                lnr = smp.tile([P, 1], f32, tag="lnr")
                nc.scalar.activation(lnr[:], ratio[:], AF.Ln)
                rk = smp.tile([P, 1], f32, tag="rk")
                nc.scalar.activation(rk[:], lnr[:], AF.Exp, scale=0.5)
                # v_ext = [v * rk | rk] per c-tile (k-side focus renorm
                # folded into v; rk column makes the kv matmul also
                # produce ksum)
                v_ext = vxp.tile([P, CT, KVW], bf, tag="vx")
                nc.vector.tensor_scalar(
                    out=v_ext[:, :, 0:P], in0=vps[:], scalar1=rk[:],
                    scalar2=None, op0=OP.mult)
                nc.vector.tensor_scalar(
                    out=v_ext[:, :, P:KVW], in0=ones_sb[:, :],
                    scalar1=rk[:], scalar2=None, op0=OP.mult)
                pending = (u3, v_ext, first, last)
                if tix == JC * JSUB - 8:
                    nc.sync.dma_start(qtile0[:], qT_v[:, :, 0:IBLK])

            # chunk-0 q-side input chain BEFORE the final kv matmuls: the
            # last kv waits ~2.5us for its u3; these 16 ready matmuls keep
            # the PE streaming through that window (and phase 2 then starts
            # a full chunk ahead).
            for nt in range(CT):
                qps0 = kpp.tile([P, IBLK], f32, tag="qps0", bufs=1)
                for ct in range(CT):
                    nc.tensor.matmul(
                        qps0[:], wq[:, ct, nt * P:(nt + 1) * P],
                        qtile0[:, ct, :],
                        start=(ct == 0), stop=(ct == CT - 1))
                rluq0 = wkp.tile([P, IBLK], bf, tag="rluq0", bufs=2)
                nc.scalar.activation(rluq0[:], qps0[:], AF.Relu)
                u2q0 = wkp.tile([P, IBLK], bf, tag="u2q0", bufs=2)
                nc.gpsimd.tensor_tensor(u2q0[:], rluq0[:], rluq0[:], OP.mult)
                nc.vector.tensor_tensor(
                    u3qs0[nt][:], u2q0[:], rluq0[:], OP.mult)
            kv_mms(pending)

            # ---------- transition: blockdiag kv, masked ksum ----------
            # split copies across DVE and ACT to halve the serial latency
            # between the last kv matmul and the first phase-2 xps matmul
            for ct in range(CT):
                half, base = ct // 2, (ct % 2) * KVW
                nc.vector.tensor_copy(
                    bd[ct][0:HD, 0:HD],
                    kv_half[half][0:HD, base:base + HD])
                nc.scalar.activation(
                    bd[ct][HD:P, HD:P],
                    kv_half[half][HD:P, base + HD:base + P], AF.Identity)
                # masked ksum: m8[c', ct, h] = ksum[c'] if head(c') == h
                nc.vector.tensor_copy(
                    m8[0:HD, ct, 2 * ct:2 * ct + 1],
                    kv_half[half][0:HD, base + P:base + P + 1])
                nc.scalar.activation(
                    m8[HD:P, ct, 2 * ct + 1:2 * ct + 2],
                    kv_half[half][HD:P, base + P:base + P + 1], AF.Identity)

        # ================= PHASE 2: q -> x -> out =================
        with ExitStack() as p2:
            # tbp first / qpsp last: PSUM banks are assigned in allocation
            # order, so qpsp lands on banks whose phase-1 readers finished
            # early — the first qps matmuls must not wait for the transition
            # copies that still read the kv banks.
            tbp = p2.enter_context(
                tc.tile_pool(name="tbps", bufs=1, space="PSUM"))
            xpsp = p2.enter_context(
                tc.tile_pool(name="xps", bufs=2, space="PSUM"))
            opsp = p2.enter_context(
                tc.tile_pool(name="ops", bufs=3, space="PSUM"))
            qpsp = p2.enter_context(
                tc.tile_pool(name="qps", bufs=2, space="PSUM"))
            ldq = p2.enter_context(tc.tile_pool(name="qld", bufs=3))
            wkq = p2.enter_context(tc.tile_pool(name="p2work", bufs=4))
            u3p = p2.enter_context(tc.tile_pool(name="u3q", bufs=13))
            xsp = p2.enter_context(tc.tile_pool(name="xs", bufs=5))
            osp = p2.enter_context(tc.tile_pool(name="osb", bufs=3))
            smq = p2.enter_context(tc.tile_pool(name="p2small", bufs=3))

            # chunk-level software pipeline: iteration ic emits the q-side
            # input chain for chunk ic and the x/out side for chunk ic-1, so
            # every PE matmul's operands are ready ~a full chunk before the
            # PE queue head reaches it.
            qtiles = {}

            def load_qtile(ic):
                if ic >= ICN:
                    return
                qt = ldq.tile([P, CT, IBLK], bf, tag="qld")
                nc.sync.dma_start(
                    qt[:], qT_v[:, :, ic * IBLK:(ic + 1) * IBLK])
                qtiles[ic] = qt

            # depth-2 pipeline: iteration ic emits t8+ln/exp for chunk ic-1,
            # the q-side input chain for chunk ic, and the x/out side for
            # chunk ic-2 — every PE matmul's operands (u3q, g8) are produced
            # a full iteration before the PE queue head reaches them.
            load_qtile(1)
            prev1 = (u3qs0, 0)  # chunk 0 produced in the phase-1 tail
            prev2 = None
            for ic in range(1, ICN + 2):
                load_qtile(ic + 1)
                # --- t8 + ln/exp for chunk ic-1 ---
                cur = None
                if prev1 is not None:
                    u3qs_1, ic_1 = prev1
                    t8 = tbp.tile([NH, IBLK], f32, tag="t8")
                    for nt in range(CT):
                        nc.tensor.matmul(
                            t8[:], m8[:, nt, :], u3qs_1[nt][:],
                            start=(nt == 0), stop=(nt == CT - 1))
                    # g8 = 1/t8 = exp(-ln(t8))  (rq cancels; eps dropped:
                    # min |t| ~ 4e2 on the real inputs)
                    lng = smq.tile([NH, IBLK], f32, tag="lng")
                    nc.scalar.activation(lng[:], t8[:], AF.Ln)
                    g8 = smq.tile([NH, IBLK], bf, tag="g8")
                    nc.scalar.activation(g8[:], lng[:], AF.Exp, scale=-1.0)
                    cur = (u3qs_1, g8, ic_1)
                # --- q-side input chain for chunk ic ---
                if ic < ICN:
                    qtile = qtiles.pop(ic)
                    u3qs = []
                    for nt in range(CT):
                        qps = qpsp.tile([P, IBLK], f32, tag="qps")
                        for ct in range(CT):
                            nc.tensor.matmul(
                                qps[:], wq[:, ct, nt * P:(nt + 1) * P],
                                qtile[:, ct, :],
                                start=(ct == 0), stop=(ct == CT - 1))
                        rluq = wkq.tile([P, IBLK], bf, tag="rluq")
                        nc.scalar.activation(rluq[:], qps[:], AF.Relu)
                        # square on ACT (queue right behind the relu),
                        # cube on GPSIMD — keeps the DVE queue free for the
                        # x/out-critical copies and multiplies
                        u2q = wkq.tile([P, IBLK], bf, tag="u2q")
                        nc.scalar.activation(u2q[:], rluq[:], AF.Square)
                        u3q = u3p.tile([P, IBLK], bf, tag="u3q")
                        nc.gpsimd.tensor_tensor(
                            u3q[:], u2q[:], rluq[:], OP.mult)
                        u3qs.append(u3q)
                # --- x / out side for chunk ic-2 ---
                if prev2 is not None:
                    u3qs_2, g8_2, ic_2 = prev2
                    isl_p = slice(ic_2 * IBLK, (ic_2 + 1) * IBLK)
                    xss = []
                    for nt in range(CT):
                        gexp_ps = opsp.tile([P, IBLK], f32, tag="ops")
                        nc.tensor.matmul(
                            gexp_ps[:], sel_sb[:, nt, :], g8_2[:],
                            start=True, stop=True)
                        gexp = wkq.tile([P, IBLK], bf, tag="gexp")
                        nc.vector.tensor_copy(gexp[:], gexp_ps[:])
                        xps = xpsp.tile([P, IBLK], f32, tag="xps")
                        nc.tensor.matmul(
                            xps[:], bd[nt][:], u3qs_2[nt][:],
                            start=True, stop=True)
                        x_s = xsp.tile([P, IBLK], bf, tag="xs")
                        nc.vector.tensor_tensor(
                            x_s[:], xps[:], gexp[:], OP.mult)
                        xss.append(x_s)

                    for et in range(CT):
                        ops_t = opsp.tile([P, IBLK], f32, tag="ops")
                        for nt in range(CT):
                            nc.tensor.matmul(
                                ops_t[:], wp[:, nt, et * P:(et + 1) * P],
                                xss[nt][:],
                                start=(nt == 0), stop=(nt == CT - 1))
                        out_sb = osp.tile([P, IBLK], f32, tag="osb")
                        nc.vector.tensor_scalar(
                            out=out_sb[:], in0=ops_t[:],
                            scalar1=bp_sb[:, et:et + 1], scalar2=None,
                            op0=OP.add)
                        nc.sync.dma_start(outT_v[:, et, isl_p], out_sb[:])
                prev2 = cur
                prev1 = (u3qs, ic) if ic < ICN else None

    nc.compile()
    return nc


def _get_nc():
    key = "nc"
    if key not in _CACHE:
        _CACHE[key] = _build_nc()
    return _CACHE[key]


def _prepare_in_maps(query, key_in, value, Wq, Wk, Wv, Wp, bp, scale):
    import ml_dtypes

    bf16 = ml_dtypes.bfloat16
    query = np.asarray(query, np.float32)
    key_in = np.asarray(key_in, np.float32)
    value = np.asarray(value, np.float32)
    Wq = np.asarray(Wq, np.float32)
    Wk = np.asarray(Wk, np.float32)
    Wv = np.asarray(Wv, np.float32)
    Wp = np.asarray(Wp, np.float32)
    bp = np.asarray(bp, np.float32)
    scale = np.asarray(scale, np.float32)

    B = query.shape[0]
    assert B == NCORES and query.shape[1] == N and query.shape[2] == C

    def rnd(a):
        return np.ascontiguousarray(np.asarray(a, np.float32).astype(bf16))

    # softplus(scale) folded into Wq/Wk columns (relu(x)/s == relu(x/s), s>0)
    s = np.log1p(np.exp(np.float64(scale.reshape(C)))).astype(np.float32)
    inv_s = (1.0 / s).astype(np.float32)
    Wq_s = rnd(Wq * inv_s[None, :])
    Wk_s = rnd(Wk * inv_s[None, :])
    Wv_r = rnd(Wv)
    Wp_r = rnd(Wp)
    bp_col = np.ascontiguousarray(bp.reshape(CT, P).T)
    ones_in = rnd(np.ones((P, CT), np.float32))
    sel8 = np.zeros((NH, CT, P), np.float32)
    for t in range(CT):
        sel8[2 * t, t, 0:HD] = 1.0
        sel8[2 * t + 1, t, HD:P] = 1.0
    sel8 = rnd(sel8)

    in_maps = []
    for b in range(B):
        in_maps.append({
            "qT": rnd(query[b].T),
            "kT": rnd(key_in[b].T),
            "vT": rnd(value[b].T),
            "Wq": Wq_s, "Wk": Wk_s, "Wv": Wv_r, "Wp": Wp_r,
            "bp_col": bp_col, "ones_in": ones_in, "sel8": sel8,
        })

    return in_maps


def kernel(query, key_in, value, Wq, Wk, Wv, Wp, bp, scale, H, W):
    from concourse.bass_utils import run_bass_kernel_spmd

    in_maps = _prepare_in_maps(
        query, key_in, value, Wq, Wk, Wv, Wp, bp, scale)
    nc = _get_nc()
    res = run_bass_kernel_spmd(nc, in_maps, list(range(NCORES)))
    out = np.empty((len(in_maps), N, C), np.float32)
    for b in range(len(in_maps)):
        out[b] = res.results[b]["outT"].T
    return out


if __name__ == "__main__":
    rng = np.random.default_rng(0)
    inputs = {
        "query": rng.standard_normal((8, N, C)).astype(np.float32),
        "key_in": rng.standard_normal((8, N, C)).astype(np.float32),
        "value": rng.standard_normal((8, N, C)).astype(np.float32),
        "Wq": (rng.standard_normal((C, C)) * 0.02).astype(np.float32),
        "Wk": (rng.standard_normal((C, C)) * 0.02).astype(np.float32),
        "Wv": (rng.standard_normal((C, C)) * 0.02).astype(np.float32),
        "Wp": (rng.standard_normal((C, C)) * 0.02).astype(np.float32),
        "bp": np.zeros((C,), np.float32),
        "scale": (rng.standard_normal((1, 1, C)) * 0.02).astype(np.float32),
        "H": 64, "W": 64,
    }
    out = kernel(**inputs)
    print("out", out.shape, out.dtype, float(np.abs(out).mean()))
